# revision 23
# baseline (speedup 1.0000x reference)
"""Trainium2 Bass kernel for AttributeRetriever top-5 cosine retrieval.

Strategy (8 NeuronCores, attr-sharded):
  - Each core owns a 1/8 shard of the attribute table (8000 rows), kept
    resident in SBUF as normalized fp16 in transposed [feat, attr] layout.
  - Selection pass: fp16 matmul image @ attr_norm.T (full 4096 batch vs own
    shard), PSUM fp32. Column indices are packed into the low 12 mantissa
    bits of the fp32 sims (4096-wide halves), so a single vector-engine
    Max8 per half yields top-8 values AND indices with no second scan.
  - Cross-core AllToAll exchanges per-image candidates; each core merges the
    8x16 candidates for its 512 output images, picks top-12 by value.
  - Exact rescore: gather the 12 raw attribute rows per image from DRAM via
    indirect DMA, recompute dot products and norms in fp32, take exact
    top-5. Output features are gathered raw rows scaled by 1/norm.
"""
import sys

for _p in ('/opt/trn_rl_repo', '/root/.axon_site/_ro/trn_rl_repo'):
    if _p not in sys.path:
        sys.path.append(_p)

import numpy as np
from contextlib import ExitStack

import concourse.bass as bass
import concourse.tile as tile
from concourse import bacc, mybir
from concourse.bass_utils import run_bass_kernel_spmd

F32 = mybir.dt.float32
F16 = mybir.dt.float16
U32 = mybir.dt.uint32
ALU = mybir.AluOpType
ACTF = mybir.ActivationFunctionType

NEG_BIG = -3.0e38


class Cfg:
    def __init__(self, B=4096, A=64000, F=512, NC=8, K=5, KP=12):
        self.B, self.A, self.F, self.NC, self.K, self.KP = B, A, F, NC, K, KP
        self.SH = A // NC              # attrs per core shard
        self.KC = F // 128             # contraction k-tiles
        self.MT = B // 128             # image m-tiles
        self.G = self.MT // NC         # m-tiles per output group
        self.HW = 4096                 # selection half width (12 idx bits)
        self.NH = (self.SH + self.HW - 1) // self.HW
        self.WIN = 512                 # matmul window (one PSUM bank)
        self.NW = (self.SH + self.WIN - 1) // self.WIN
        self.NCAND = self.NH * 8       # candidates per core per image
        self.MRG = NC * self.NCAND     # merged candidates per image
        assert self.B % (128 * NC) == 0 and self.A % NC == 0
        assert self.HW % self.WIN == 0 and self.F % 128 == 0
        assert self.KP >= K and self.KP % 4 == 0 and self.MRG <= 4096


def build(cfg: Cfg):
    c = cfg
    nc = bacc.Bacc("TRN2", target_bir_lowering=False, debug=False,
                   num_devices=c.NC)

    attr_t = nc.dram_tensor("attr_t", [c.F, c.SH], F32, kind="ExternalInput").ap()
    attr_n = nc.dram_tensor("attr_n", [c.SH, c.F], F32, kind="ExternalInput").ap()
    attr_full = nc.dram_tensor("attr_full", [c.NC * 8192, c.F], F32, kind="ExternalInput").ap()
    img_t = nc.dram_tensor("img_t", [c.MT, 128, c.KC * 128], F32, kind="ExternalInput").ap()
    img_rows = nc.dram_tensor("img_rows", [c.G * 128, c.F], F32, kind="ExternalInput").ap()
    hbase_d = nc.dram_tensor("hbase", [128, c.NH], F32, kind="ExternalInput").ap()

    o_feat = nc.dram_tensor("o_feat", [c.G * 128, c.K, c.F], F32, kind="ExternalOutput").ap()
    o_scores = nc.dram_tensor("o_scores", [c.G * 128, c.K], F32, kind="ExternalOutput").ap()

    NB = (c.SH + 511) // 512   # natural-layout norm blocks: 4 rows/partition

    with tile.TileContext(nc) as tc, ExitStack() as ctx:
        pool = ctx.enter_context(tc.tile_pool(name="main", bufs=1))
        wpool = ctx.enter_context(tc.tile_pool(name="work", bufs=2))
        spool = ctx.enter_context(tc.tile_pool(name="small", bufs=2))
        tpool = ctx.enter_context(tc.tile_pool(name="tail", bufs=2))
        psum = ctx.enter_context(tc.tile_pool(name="ps", bufs=2, space="PSUM"))
        dram = ctx.enter_context(tc.tile_pool(name="dr", bufs=1, space="DRAM"))

        # ---- constants ----
        iota_t = pool.tile([128, c.HW], U32, tag="iota")
        nc.gpsimd.iota(iota_t[:], pattern=[[1, c.HW]], base=0, channel_multiplier=0)
        m_hi20 = pool.tile([128, 1], U32, tag="m_hi20")   # clear low 13 bits
        nc.vector.memset(m_hi20[:], 0xFFFFE000)
        m_h12 = pool.tile([128, 1], U32, tag="m_h12")     # half offset bit (4096)
        nc.vector.memset(m_h12[:], 0x00001000)
        m_hi24 = pool.tile([128, 1], U32, tag="m_hi24")   # clear low 8 bits
        nc.vector.memset(m_hi24[:], 0xFFFFFF00)
        m_lo8 = pool.tile([128, 1], U32, tag="m_lo8")
        nc.vector.memset(m_lo8[:], 0x000000FF)
        m_lo16 = pool.tile([128, 1], U32, tag="m_lo16")
        nc.vector.memset(m_lo16[:], 0x0000FFFF)
        hbase = pool.tile([128, c.NH], F32, tag="hbase")  # shard base + h*HW
        nc.sync.dma_start(hbase[:], hbase_d[:])
        iota_f = pool.tile([128, max(c.MRG, 16)], F32, tag="iota_f")
        nc.vector.tensor_copy(iota_f[:], iota_t[:, :max(c.MRG, 16)])

        # ---- phase A: attr norms from natural layout (4 rows/partition) ----
        NBH = (NB + 1) // 2
        rb = dram.tile([NB, 128, 4], F32, tag="rinv_bounce")
        rb_flat = rb[:].rearrange("b p i -> (b p i)").unsqueeze(0)
        for half in range(2):
            b0 = half * NBH
            b1 = min(NB, b0 + NBH)
            nbh = b1 - b0
            if nbh <= 0:
                continue
            ssq_nat = pool.tile([128, nbh * 4], F32, tag=f"ssq_nat{half}",
                                name=f"ssq_nat{half}")
            nc.vector.memset(ssq_nat[:], 1.0)
            for b in range(b0, b1):
                rows = min(512, c.SH - b * 512)
                pt = rows // 4
                nat = wpool.tile([128, 4, c.F], F32, tag="nat")
                nc.sync.dma_start(
                    nat[:pt, :, :],
                    attr_n[b * 512: b * 512 + rows, :].rearrange(
                        "(p i) f -> p i f", i=4))
                sq16 = wpool.tile([128, c.F], F16, tag="sq16")
                for i in range(4):
                    nc.scalar.activation(
                        sq16[:pt, :], nat[:pt, i, :], ACTF.Square,
                        accum_out=ssq_nat[:pt, (b - b0) * 4 + i: (b - b0) * 4 + i + 1])
            nrm_nat = pool.tile([128, nbh * 4], F32, tag=f"nrm_nat{half}",
                                name=f"nrm_nat{half}")
            nc.scalar.activation(nrm_nat[:], ssq_nat[:], ACTF.Sqrt)
            rinv_nat = pool.tile([128, nbh * 4], F32, tag=f"rinv_nat{half}",
                                 name=f"rinv_nat{half}")
            nc.vector.reciprocal(rinv_nat[:], nrm_nat[:])
            nc.sync.dma_start(
                rb[b0:b1].rearrange("b p i -> p b i"),
                rinv_nat[:].rearrange("p (b i) -> p b i", i=4))

        # ---- phase B: load + normalize rhs shard (transposed), fp16 ----
        NCH = (c.SH + 2047) // 2048     # 2048-wide attr chunks
        attrc = []
        for ch in range(NCH):
            cw = min(2048, c.SH - ch * 2048)
            ac = pool.tile([128, c.KC, cw], F16, tag=f"attrc{ch}", name=f"attrc{ch}")
            attrc.append(ac)
        for ch in range(NCH):
            cw = min(2048, c.SH - ch * 2048)
            scale_c = wpool.tile([128, 2048], F32, tag="scale_c")
            rslice = pool.tile([1, 2048], F32, tag="rslice")
            nc.sync.dma_start(rslice[0:1, :cw],
                              rb_flat[:, ch * 2048: ch * 2048 + cw])
            nc.gpsimd.partition_broadcast(scale_c[:, :cw], rslice[0:1, :cw])
            for kc in range(c.KC):
                stg = wpool.tile([128, 2048], F32, tag="stg")
                nc.sync.dma_start(
                    stg[:, :cw],
                    attr_t[kc * 128:(kc + 1) * 128, ch * 2048: ch * 2048 + cw])
                nc.vector.tensor_tensor(
                    attrc[ch][:, kc, :cw], stg[:, :cw], scale_c[:, :cw],
                    op=ALU.mult)

        # ---- phase D: selection matmuls + packed top-8 per half ----
        # candidate = (value & 0xFFFF0000) | global_attr_idx16
        cand16 = pool.tile([128, c.MT, c.NCAND], F32, tag="cand16")
        m_hi16 = pool.tile([128, 1], U32, tag="m_hi16")
        nc.vector.memset(m_hi16[:], 0xFFFF0000)
        for m in range(c.MT):
            imgf = wpool.tile([128, c.KC, 128], F32, tag="imgf")
            nc.sync.dma_start(imgf[:], img_t[m].rearrange("p (kc j) -> p kc j", kc=c.KC))
            imgh = wpool.tile([128, c.KC, 128], F16, tag="imgh")
            nc.scalar.copy(imgh[:], imgf[:])
            for h in range(c.NH):
                hw = min(c.HW, c.SH - h * c.HW)
                packed = pool.tile([128, c.HW], U32, tag="packed")
                for ql in range(2):   # two PSUM tiles per half
                    q0 = ql * (c.HW // 2)
                    qw = min(c.HW // 2, hw - q0)
                    if qw <= 0:
                        break
                    ch = h * 2 + ql
                    ps = psum.tile([128, c.HW // 2], F32, tag="mainps")
                    for wq in range((qw + c.WIN - 1) // c.WIN):
                        woff = wq * c.WIN
                        nw = min(c.WIN, qw - woff)
                        for kc in range(c.KC):
                            nc.tensor.matmul(
                                ps[:, woff: woff + nw],
                                imgh[:, kc, :],
                                attrc[ch][:, kc, woff: woff + nw],
                                start=(kc == 0), stop=(kc == c.KC - 1))
                    # pack directly from PSUM: (sim & hi20) | iota(q0..q0+qw)
                    nc.vector.scalar_tensor_tensor(
                        packed[:, q0: q0 + qw], ps[:, :qw].bitcast(U32),
                        m_hi20[:], iota_t[:, q0: q0 + qw],
                        op0=ALU.bitwise_and, op1=ALU.bitwise_or)
                nc.vector.max(cand16[:, m, h * 8:(h + 1) * 8],
                              packed[:, :hw].bitcast(F32))

        # ---- phase E: clear bits 13-15 (room for sender slot), set half bit ----
        m_c1315 = pool.tile([128, 1], U32, tag="m_c1315")
        nc.vector.memset(m_c1315[:], 0xFFFF1FFF)
        call = cand16[:].bitcast(U32).rearrange("p mt k -> p (mt k)")
        nc.vector.tensor_scalar(call, call, m_c1315[:], None, op0=ALU.bitwise_and)
        cview = cand16[:].rearrange("p mt (h k) -> p mt h k", h=c.NH)
        for h in range(1, c.NH):
            sl = cview[:, :, h, :].bitcast(U32)
            nc.vector.tensor_scalar(sl, sl, m_h12[:], None, op0=ALU.bitwise_or)

        # ---- phase E2: AllToAll exchange of candidates ----
        a2a_in = dram.tile([c.NC, 128, c.G, c.NCAND], F32, tag="a2a_in")
        a2a_out = dram.tile([c.NC, 128, c.G, c.NCAND], F32, tag="a2a_out")
        for g in range(c.NC):
            nc.sync.dma_start(
                a2a_in[g].rearrange("p j k -> p j k"),
                cand16[:, g * c.G:(g + 1) * c.G, :])
        nc.gpsimd.collective_compute(
            "AllToAll", ALU.bypass, replica_groups=[list(range(c.NC))],
            ins=[a2a_in.opt()], outs=[a2a_out.opt()])

        # per-slot sender base pattern: value = cc << 13 for slot cc
        ccbase = pool.tile([128, c.NC, c.NCAND], U32, tag="ccbase")
        nc.gpsimd.iota(ccbase[:], pattern=[[8192, c.NC], [0, c.NCAND]],
                       base=0, channel_multiplier=0)
        # ---- phase F/G: per output m-tile: merge, top-KP, rescore, top-5 ----
        for j in range(c.G):
            mv = spool.tile([128, c.MRG], F32, tag="mv")
            for cc in range(c.NC):
                nc.sync.dma_start(mv[:, cc * c.NCAND:(cc + 1) * c.NCAND],
                                  a2a_out[cc, :, j, :])
            nc.vector.tensor_tensor(
                mv[:].bitcast(U32),
                mv[:].bitcast(U32),
                ccbase[:].rearrange("p a b -> p (a b)"), op=ALU.bitwise_or)

            # top-KP by packed16 value (idx16 rides in the low bits)
            nrounds = (c.KP + 7) // 8
            sel16 = spool.tile([128, nrounds * 8], F32, tag="sel16")
            cur = mv
            for rnd in range(nrounds):
                t8 = spool.tile([128, 8], F32, tag="t8sel")
                nc.vector.max(t8[:], cur[:])
                nc.vector.tensor_copy(sel16[:, rnd * 8:(rnd + 1) * 8], t8[:])
                if rnd + 1 < nrounds:
                    nxt = spool.tile([128, c.MRG], F32, tag="pk_nxt")
                    nc.vector.match_replace(nxt[:], t8[:], cur[:], NEG_BIG)
                    cur = nxt
            cidx_u = spool.tile([128, c.KP], U32, tag="cidx_u")
            nc.vector.tensor_scalar(cidx_u[:], sel16[:, :c.KP].bitcast(U32), m_lo16[:],
                                    None, op0=ALU.bitwise_and)
            cidx = spool.tile([128, c.KP], F32, tag="cidx")
            nc.vector.tensor_copy(cidx[:], cidx_u[:])

            # gather raw attr rows + rescore, in batches to bound SBUF
            imgr = spool.tile([128, c.F], F32, tag="imgr")
            nc.sync.dma_start(imgr[:], img_rows[j * 128:(j + 1) * 128, :])
            HP = 4
            dots = spool.tile([128, c.KP], F32, tag="dots")
            ssq = spool.tile([128, c.KP], F32, tag="ssq")
            sqs = spool.tile([128, c.F], F16, tag="sqs")
            for hh in range(c.KP // HP):
                gvec = tpool.tile([128, HP, c.F], F32, tag="gvec")
                for s in range(HP):
                    sl = hh * HP + s
                    nc.gpsimd.indirect_dma_start(
                        out=gvec[:, s, :], out_offset=None, in_=attr_full[:],
                        in_offset=bass.IndirectOffsetOnAxis(
                            ap=cidx_u[:, sl:sl + 1], axis=0))
                prod = pool.tile([128, HP, c.F], F32, tag="prod")
                nc.vector.tensor_tensor(
                    prod[:], gvec[:],
                    imgr[:].unsqueeze(1).broadcast_to([128, HP, c.F]),
                    op=ALU.mult)
                nc.vector.tensor_reduce(dots[:, hh * HP:(hh + 1) * HP], prod[:],
                                        op=ALU.add, axis=mybir.AxisListType.X)
                for s in range(HP):
                    nc.scalar.activation(sqs[:], gvec[:, s, :], ACTF.Square,
                                         accum_out=ssq[:, hh * HP + s: hh * HP + s + 1])
            nrm = spool.tile([128, c.KP], F32, tag="nrm")
            nc.scalar.activation(nrm[:], ssq[:], ACTF.Sqrt)
            rinv = spool.tile([128, c.KP], F32, tag="rinv")
            nc.vector.reciprocal(rinv[:], nrm[:])
            score = spool.tile([128, c.KP], F32, tag="score")
            nc.vector.tensor_tensor(score[:], dots[:], rinv[:], op=ALU.mult)

            # exact top-5 with indices
            t8f = spool.tile([128, 8], F32, tag="t8f")
            nc.vector.max(t8f[:], score[:])
            pos8 = spool.tile([128, 8], U32, tag="pos8")
            nc.vector.max_index(pos8[:], t8f[:], score[:])
            pos8_f = spool.tile([128, 8], F32, tag="pos8_f")
            nc.vector.tensor_copy(pos8_f[:], pos8[:])
            widx = spool.tile([128, c.K], F32, tag="widx")
            mks = spool.tile([128, c.KP], F32, tag="mks")
            mksd = spool.tile([128, c.KP], F32, tag="mksd")
            for k in range(c.K):
                nc.vector.tensor_scalar(mks[:], iota_f[:, :c.KP],
                                        pos8_f[:, k:k + 1], None, op0=ALU.is_equal)
                nc.vector.tensor_tensor(mksd[:], mks[:], cidx[:], op=ALU.mult)
                nc.vector.tensor_reduce(widx[:, k:k + 1], mksd[:],
                                        op=ALU.add, axis=mybir.AxisListType.X)
            widx_u = spool.tile([128, c.K], U32, tag="widx_u")
            nc.vector.tensor_copy(widx_u[:], widx[:])
            nc.sync.dma_start(o_scores[j * 128:(j + 1) * 128, :], t8f[:, :c.K])

            # final gather of winner rows + exact normalize -> features
            g2 = pool.tile([128, c.K, c.F], F32, tag="g2")
            for k in range(c.K):
                nc.gpsimd.indirect_dma_start(
                    out=g2[:, k, :], out_offset=None, in_=attr_full[:],
                    in_offset=bass.IndirectOffsetOnAxis(ap=widx_u[:, k:k + 1], axis=0))
            ssq2 = spool.tile([128, c.K], F32, tag="ssq2")
            for k in range(c.K):
                nc.scalar.activation(sqs[:], g2[:, k, :], ACTF.Square,
                                     accum_out=ssq2[:, k:k + 1])
            nrm2 = spool.tile([128, c.K], F32, tag="nrm2")
            nc.scalar.activation(nrm2[:], ssq2[:], ACTF.Sqrt)
            rinv2 = spool.tile([128, c.K], F32, tag="rinv2")
            nc.vector.reciprocal(rinv2[:], nrm2[:])
            for k in range(c.K):
                nc.vector.tensor_scalar(g2[:, k, :], g2[:, k, :],
                                        rinv2[:, k:k + 1], None, op0=ALU.mult)
            nc.sync.dma_start(
                o_feat[j * 128:(j + 1) * 128, :, :].rearrange("p k f -> p (k f)"),
                g2[:].rearrange("p k f -> p (k f)"))

    nc.compile()
    return nc


_BUILT = {}


def _get_built(cfg: Cfg):
    key = (cfg.B, cfg.A, cfg.F, cfg.NC, cfg.K, cfg.KP)
    if key not in _BUILT:
        _BUILT[key] = build(cfg)
    return _BUILT[key]


def run(image_features, attr_features, cfg: Cfg, trace=False, **kw):
    c = cfg
    img = np.ascontiguousarray(np.asarray(image_features, dtype=np.float32))
    attr = np.ascontiguousarray(np.asarray(attr_features, dtype=np.float32))
    assert img.shape == (c.B, c.F) and attr.shape == (c.A, c.F)

    nc = _get_built(cfg)
    attr_pad = np.zeros((c.NC * 8192, c.F), dtype=np.float32)
    for r in range(c.NC):
        attr_pad[r * 8192: r * 8192 + c.SH] = attr[r * c.SH:(r + 1) * c.SH]
    # [MT, p(feat within kc), kc, j(img within tile)] contiguous per-partition
    img_tiled = np.ascontiguousarray(
        img.reshape(c.MT, 128, c.KC, 128).transpose(0, 3, 2, 1)
    ).reshape(c.MT, 128, c.KC * 128)
    attr_T = np.ascontiguousarray(attr.T)
    in_maps = []
    for r in range(c.NC):
        s0 = r * c.SH
        hb = (s0 + np.arange(c.NH, dtype=np.float32) * c.HW)[None, :].astype(np.float32)
        in_maps.append({
            "attr_t": np.ascontiguousarray(attr_T[:, s0:s0 + c.SH]),
            "attr_n": np.ascontiguousarray(attr[s0:s0 + c.SH]),
            "attr_full": attr_pad,
            "img_t": img_tiled,
            "img_rows": np.ascontiguousarray(img[r * c.G * 128:(r + 1) * c.G * 128]),
            "hbase": np.ascontiguousarray(np.repeat(hb, 128, axis=0)),
        })
    try:
        res = run_bass_kernel_spmd(nc, in_maps, core_ids=list(range(c.NC)),
                                   trace=trace, **kw)
    except Exception:
        # transient NRT_EXEC_UNIT_UNRECOVERABLE wedges recover on retry
        import time as _time
        _time.sleep(2.0)
        res = run_bass_kernel_spmd(nc, in_maps, core_ids=list(range(c.NC)),
                                   trace=trace, **kw)
    feat = np.concatenate([res.results[r]["o_feat"] for r in range(c.NC)], axis=0)
    scores = np.concatenate([res.results[r]["o_scores"] for r in range(c.NC)], axis=0)
    return (feat, scores), res


def kernel(image_features, attr_features):
    (feat, scores), _ = run(image_features, attr_features, Cfg())
    return (feat, scores)


# revision 24
# speedup vs baseline: 1.0041x; 1.0041x over previous
"""Trainium2 Bass kernel for AttributeRetriever top-5 cosine retrieval.

Strategy (8 NeuronCores, attr-sharded):
  - Each core owns a 1/8 shard of the attribute table (8000 rows), kept
    resident in SBUF as normalized fp16 in transposed [feat, attr] layout.
  - Selection pass: fp16 matmul image @ attr_norm.T (full 4096 batch vs own
    shard), PSUM fp32. Column indices are packed into the low 12 mantissa
    bits of the fp32 sims (4096-wide halves), so a single vector-engine
    Max8 per half yields top-8 values AND indices with no second scan.
  - Cross-core AllToAll exchanges per-image candidates; each core merges the
    8x16 candidates for its 512 output images, picks top-12 by value.
  - Exact rescore: gather the 12 raw attribute rows per image from DRAM via
    indirect DMA, recompute dot products and norms in fp32, take exact
    top-5. Output features are gathered raw rows scaled by 1/norm.
"""
import sys

for _p in ('/opt/trn_rl_repo', '/root/.axon_site/_ro/trn_rl_repo'):
    if _p not in sys.path:
        sys.path.append(_p)

import numpy as np
from contextlib import ExitStack

import concourse.bass as bass
import concourse.tile as tile
from concourse import bacc, mybir
from concourse.bass_utils import run_bass_kernel_spmd

F32 = mybir.dt.float32
F16 = mybir.dt.float16
U32 = mybir.dt.uint32
ALU = mybir.AluOpType
ACTF = mybir.ActivationFunctionType

NEG_BIG = -3.0e38


class Cfg:
    def __init__(self, B=4096, A=64000, F=512, NC=8, K=5, KP=12):
        self.B, self.A, self.F, self.NC, self.K, self.KP = B, A, F, NC, K, KP
        self.SH = A // NC              # attrs per core shard
        self.KC = F // 128             # contraction k-tiles
        self.MT = B // 128             # image m-tiles
        self.G = self.MT // NC         # m-tiles per output group
        self.HW = 4096                 # selection half width (12 idx bits)
        self.NH = (self.SH + self.HW - 1) // self.HW
        self.WIN = 512                 # matmul window (one PSUM bank)
        self.NW = (self.SH + self.WIN - 1) // self.WIN
        self.NCAND = self.NH * 8       # candidates per core per image
        self.MRG = NC * self.NCAND     # merged candidates per image
        assert self.B % (128 * NC) == 0 and self.A % NC == 0
        assert self.HW % self.WIN == 0 and self.F % 128 == 0
        assert self.KP >= K and self.KP % 4 == 0 and self.MRG <= 4096


def build(cfg: Cfg):
    c = cfg
    nc = bacc.Bacc("TRN2", target_bir_lowering=False, debug=False,
                   num_devices=c.NC)

    attr_t = nc.dram_tensor("attr_t", [c.F, c.SH], F32, kind="ExternalInput").ap()
    attr_n = nc.dram_tensor("attr_n", [c.SH, c.F], F32, kind="ExternalInput").ap()
    attr_full = nc.dram_tensor("attr_full", [c.NC * 8192, c.F], F32, kind="ExternalInput").ap()
    img_t = nc.dram_tensor("img_t", [c.MT, 128, c.KC * 128], F32, kind="ExternalInput").ap()
    img_rows = nc.dram_tensor("img_rows", [c.G * 128, c.F], F32, kind="ExternalInput").ap()
    hbase_d = nc.dram_tensor("hbase", [128, c.NH], F32, kind="ExternalInput").ap()

    o_feat = nc.dram_tensor("o_feat", [c.G * 128, c.K, c.F], F32, kind="ExternalOutput").ap()
    o_scores = nc.dram_tensor("o_scores", [c.G * 128, c.K], F32, kind="ExternalOutput").ap()

    NB = (c.SH + 511) // 512   # natural-layout norm blocks: 4 rows/partition

    with tile.TileContext(nc) as tc, ExitStack() as ctx:
        pool = ctx.enter_context(tc.tile_pool(name="main", bufs=1))
        wpool = ctx.enter_context(tc.tile_pool(name="work", bufs=2))
        spool = ctx.enter_context(tc.tile_pool(name="small", bufs=2))
        tpool = ctx.enter_context(tc.tile_pool(name="tail", bufs=2))
        psum = ctx.enter_context(tc.tile_pool(name="ps", bufs=2, space="PSUM"))
        dram = ctx.enter_context(tc.tile_pool(name="dr", bufs=1, space="DRAM"))

        # ---- constants ----
        iota_t = pool.tile([128, c.HW], U32, tag="iota")
        nc.gpsimd.iota(iota_t[:], pattern=[[1, c.HW]], base=0, channel_multiplier=0)
        m_hi20 = pool.tile([128, 1], U32, tag="m_hi20")   # clear low 13 bits
        nc.vector.memset(m_hi20[:], 0xFFFFE000)
        m_h12 = pool.tile([128, 1], U32, tag="m_h12")     # half offset bit (4096)
        nc.vector.memset(m_h12[:], 0x00001000)
        m_hi24 = pool.tile([128, 1], U32, tag="m_hi24")   # clear low 8 bits
        nc.vector.memset(m_hi24[:], 0xFFFFFF00)
        m_lo8 = pool.tile([128, 1], U32, tag="m_lo8")
        nc.vector.memset(m_lo8[:], 0x000000FF)
        m_lo16 = pool.tile([128, 1], U32, tag="m_lo16")
        nc.vector.memset(m_lo16[:], 0x0000FFFF)
        hbase = pool.tile([128, c.NH], F32, tag="hbase")  # shard base + h*HW
        nc.sync.dma_start(hbase[:], hbase_d[:])
        iota_f = pool.tile([128, max(c.MRG, 16)], F32, tag="iota_f")
        nc.vector.tensor_copy(iota_f[:], iota_t[:, :max(c.MRG, 16)])

        # ---- phase A: attr norms from natural layout (4 rows/partition) ----
        NBH = (NB + 1) // 2
        rb = dram.tile([NB, 128, 4], F32, tag="rinv_bounce")
        rb_flat = rb[:].rearrange("b p i -> (b p i)").unsqueeze(0)
        for half in range(2):
            b0 = half * NBH
            b1 = min(NB, b0 + NBH)
            nbh = b1 - b0
            if nbh <= 0:
                continue
            ssq_nat = pool.tile([128, nbh * 4], F32, tag=f"ssq_nat{half}",
                                name=f"ssq_nat{half}")
            nc.vector.memset(ssq_nat[:], 1.0)
            for b in range(b0, b1):
                rows = min(512, c.SH - b * 512)
                pt = rows // 4
                nat = wpool.tile([128, 4, c.F], F32, tag="nat")
                nc.sync.dma_start(
                    nat[:pt, :, :],
                    attr_n[b * 512: b * 512 + rows, :].rearrange(
                        "(p i) f -> p i f", i=4))
                sq16 = wpool.tile([128, c.F], F16, tag="sq16")
                for i in range(4):
                    nc.scalar.activation(
                        sq16[:pt, :], nat[:pt, i, :], ACTF.Square,
                        accum_out=ssq_nat[:pt, (b - b0) * 4 + i: (b - b0) * 4 + i + 1])
            nrm_nat = pool.tile([128, nbh * 4], F32, tag=f"nrm_nat{half}",
                                name=f"nrm_nat{half}")
            nc.scalar.activation(nrm_nat[:], ssq_nat[:], ACTF.Sqrt)
            rinv_nat = pool.tile([128, nbh * 4], F32, tag=f"rinv_nat{half}",
                                 name=f"rinv_nat{half}")
            nc.vector.reciprocal(rinv_nat[:], nrm_nat[:])
            nc.sync.dma_start(
                rb[b0:b1].rearrange("b p i -> p b i"),
                rinv_nat[:].rearrange("p (b i) -> p b i", i=4))

        # ---- phase B: load + normalize rhs shard (transposed), fp16 ----
        NCH = (c.SH + 2047) // 2048     # 2048-wide attr chunks
        attrc = []
        for ch in range(NCH):
            cw = min(2048, c.SH - ch * 2048)
            ac = pool.tile([128, c.KC, cw], F16, tag=f"attrc{ch}", name=f"attrc{ch}")
            attrc.append(ac)
        for ch in range(NCH):
            cw = min(2048, c.SH - ch * 2048)
            scale_c = wpool.tile([128, 2048], F32, tag="scale_c")
            rslice = pool.tile([1, 2048], F32, tag="rslice")
            nc.sync.dma_start(rslice[0:1, :cw],
                              rb_flat[:, ch * 2048: ch * 2048 + cw])
            nc.gpsimd.partition_broadcast(scale_c[:, :cw], rslice[0:1, :cw])
            for kc in range(c.KC):
                stg = wpool.tile([128, 2048], F32, tag="stg")
                nc.sync.dma_start(
                    stg[:, :cw],
                    attr_t[kc * 128:(kc + 1) * 128, ch * 2048: ch * 2048 + cw])
                nc.vector.tensor_tensor(
                    attrc[ch][:, kc, :cw], stg[:, :cw], scale_c[:, :cw],
                    op=ALU.mult)

        # ---- phase D: selection matmuls + packed top-8 per half ----
        # candidate = (value & 0xFFFF0000) | idx16; one tile per output group
        cand16g = []
        for g in range(c.NC):
            cg = pool.tile([128, c.G, c.NCAND], F32, tag=f"cand16g{g}",
                           name=f"cand16g{g}")
            cand16g.append(cg)
        for m in range(c.MT):
            imgf = wpool.tile([128, c.KC, 128], F32, tag="imgf")
            nc.sync.dma_start(imgf[:], img_t[m].rearrange("p (kc j) -> p kc j", kc=c.KC))
            imgh = wpool.tile([128, c.KC, 128], F16, tag="imgh")
            nc.scalar.copy(imgh[:], imgf[:])
            for h in range(c.NH):
                hw = min(c.HW, c.SH - h * c.HW)
                packed = pool.tile([128, c.HW], U32, tag="packed")
                for ql in range(2):   # two PSUM tiles per half
                    q0 = ql * (c.HW // 2)
                    qw = min(c.HW // 2, hw - q0)
                    if qw <= 0:
                        break
                    ch = h * 2 + ql
                    ps = psum.tile([128, c.HW // 2], F32, tag="mainps")
                    for wq in range((qw + c.WIN - 1) // c.WIN):
                        woff = wq * c.WIN
                        nw = min(c.WIN, qw - woff)
                        for kc in range(c.KC):
                            nc.tensor.matmul(
                                ps[:, woff: woff + nw],
                                imgh[:, kc, :],
                                attrc[ch][:, kc, woff: woff + nw],
                                start=(kc == 0), stop=(kc == c.KC - 1))
                    # pack directly from PSUM: (sim & hi20) | iota(q0..q0+qw)
                    nc.vector.scalar_tensor_tensor(
                        packed[:, q0: q0 + qw], ps[:, :qw].bitcast(U32),
                        m_hi20[:], iota_t[:, q0: q0 + qw],
                        op0=ALU.bitwise_and, op1=ALU.bitwise_or)
                nc.vector.max(cand16g[m // c.G][:, m % c.G, h * 8:(h + 1) * 8],
                              packed[:, :hw].bitcast(F32))

        # ---- phase E: per-group bit fixup + a2a-in DMA (overlap main loop) ----
        m_c1315 = pool.tile([128, 1], U32, tag="m_c1315")
        nc.vector.memset(m_c1315[:], 0xFFFF1FFF)
        a2a_in = dram.tile([c.NC, 128, c.G, c.NCAND], F32, tag="a2a_in")
        a2a_out = dram.tile([c.NC, 128, c.G, c.NCAND], F32, tag="a2a_out")
        for g in range(c.NC):
            cg = cand16g[g]
            call = cg[:].bitcast(U32).rearrange("p j k -> p (j k)")
            nc.vector.tensor_scalar(call, call, m_c1315[:], None,
                                    op0=ALU.bitwise_and)
            for h in range(1, c.NH):
                sl = cg[:, :, h * 8:(h + 1) * 8].bitcast(U32)
                nc.vector.tensor_scalar(sl, sl, m_h12[:], None,
                                        op0=ALU.bitwise_or)
            nc.sync.dma_start(a2a_in[g], cg[:])
        nc.gpsimd.collective_compute(
            "AllToAll", ALU.bypass, replica_groups=[list(range(c.NC))],
            ins=[a2a_in.opt()], outs=[a2a_out.opt()])

        # per-slot sender base pattern: value = cc << 13 for slot cc
        ccbase = pool.tile([128, c.NC, c.NCAND], U32, tag="ccbase")
        nc.gpsimd.iota(ccbase[:], pattern=[[8192, c.NC], [0, c.NCAND]],
                       base=0, channel_multiplier=0)
        # ---- phase F/G: per output m-tile: merge, top-KP, rescore, top-5 ----
        for j in range(c.G):
            mv = spool.tile([128, c.MRG], F32, tag="mv")
            for cc in range(c.NC):
                nc.sync.dma_start(mv[:, cc * c.NCAND:(cc + 1) * c.NCAND],
                                  a2a_out[cc, :, j, :])
            nc.vector.tensor_tensor(
                mv[:].bitcast(U32),
                mv[:].bitcast(U32),
                ccbase[:].rearrange("p a b -> p (a b)"), op=ALU.bitwise_or)

            # top-KP by packed16 value (idx16 rides in the low bits)
            nrounds = (c.KP + 7) // 8
            sel16 = spool.tile([128, nrounds * 8], F32, tag="sel16")
            cur = mv
            for rnd in range(nrounds):
                t8 = spool.tile([128, 8], F32, tag="t8sel")
                nc.vector.max(t8[:], cur[:])
                nc.vector.tensor_copy(sel16[:, rnd * 8:(rnd + 1) * 8], t8[:])
                if rnd + 1 < nrounds:
                    nxt = spool.tile([128, c.MRG], F32, tag="pk_nxt")
                    nc.vector.match_replace(nxt[:], t8[:], cur[:], NEG_BIG)
                    cur = nxt
            cidx_u = spool.tile([128, c.KP], U32, tag="cidx_u")
            nc.vector.tensor_scalar(cidx_u[:], sel16[:, :c.KP].bitcast(U32), m_lo16[:],
                                    None, op0=ALU.bitwise_and)
            cidx = spool.tile([128, c.KP], F32, tag="cidx")
            nc.vector.tensor_copy(cidx[:], cidx_u[:])

            # gather raw attr rows + rescore, in batches to bound SBUF
            imgr = spool.tile([128, c.F], F32, tag="imgr")
            nc.sync.dma_start(imgr[:], img_rows[j * 128:(j + 1) * 128, :])
            HP = 4
            dots = spool.tile([128, c.KP], F32, tag="dots")
            ssq = spool.tile([128, c.KP], F32, tag="ssq")
            sqs = spool.tile([128, c.F], F16, tag="sqs")
            for hh in range(c.KP // HP):
                gvec = tpool.tile([128, HP, c.F], F32, tag="gvec")
                for s in range(HP):
                    sl = hh * HP + s
                    nc.gpsimd.indirect_dma_start(
                        out=gvec[:, s, :], out_offset=None, in_=attr_full[:],
                        in_offset=bass.IndirectOffsetOnAxis(
                            ap=cidx_u[:, sl:sl + 1], axis=0))
                prod = pool.tile([128, HP, c.F], F32, tag="prod")
                nc.vector.tensor_tensor(
                    prod[:], gvec[:],
                    imgr[:].unsqueeze(1).broadcast_to([128, HP, c.F]),
                    op=ALU.mult)
                nc.vector.tensor_reduce(dots[:, hh * HP:(hh + 1) * HP], prod[:],
                                        op=ALU.add, axis=mybir.AxisListType.X)
                for s in range(HP):
                    nc.scalar.activation(sqs[:], gvec[:, s, :], ACTF.Square,
                                         accum_out=ssq[:, hh * HP + s: hh * HP + s + 1])
            nrm = spool.tile([128, c.KP], F32, tag="nrm")
            nc.scalar.activation(nrm[:], ssq[:], ACTF.Sqrt)
            rinv = spool.tile([128, c.KP], F32, tag="rinv")
            nc.vector.reciprocal(rinv[:], nrm[:])
            score = spool.tile([128, c.KP], F32, tag="score")
            nc.vector.tensor_tensor(score[:], dots[:], rinv[:], op=ALU.mult)

            # exact top-5 with indices
            t8f = spool.tile([128, 8], F32, tag="t8f")
            nc.vector.max(t8f[:], score[:])
            pos8 = spool.tile([128, 8], U32, tag="pos8")
            nc.vector.max_index(pos8[:], t8f[:], score[:])
            pos8_f = spool.tile([128, 8], F32, tag="pos8_f")
            nc.vector.tensor_copy(pos8_f[:], pos8[:])
            widx = spool.tile([128, c.K], F32, tag="widx")
            mks = spool.tile([128, c.KP], F32, tag="mks")
            mksd = spool.tile([128, c.KP], F32, tag="mksd")
            for k in range(c.K):
                nc.vector.tensor_scalar(mks[:], iota_f[:, :c.KP],
                                        pos8_f[:, k:k + 1], None, op0=ALU.is_equal)
                nc.vector.tensor_tensor(mksd[:], mks[:], cidx[:], op=ALU.mult)
                nc.vector.tensor_reduce(widx[:, k:k + 1], mksd[:],
                                        op=ALU.add, axis=mybir.AxisListType.X)
            widx_u = spool.tile([128, c.K], U32, tag="widx_u")
            nc.vector.tensor_copy(widx_u[:], widx[:])
            nc.sync.dma_start(o_scores[j * 128:(j + 1) * 128, :], t8f[:, :c.K])

            # final gather of winner rows + exact normalize -> features
            g2 = pool.tile([128, c.K, c.F], F32, tag="g2")
            for k in range(c.K):
                nc.gpsimd.indirect_dma_start(
                    out=g2[:, k, :], out_offset=None, in_=attr_full[:],
                    in_offset=bass.IndirectOffsetOnAxis(ap=widx_u[:, k:k + 1], axis=0))
            ssq2 = spool.tile([128, c.K], F32, tag="ssq2")
            for k in range(c.K):
                nc.scalar.activation(sqs[:], g2[:, k, :], ACTF.Square,
                                     accum_out=ssq2[:, k:k + 1])
            nrm2 = spool.tile([128, c.K], F32, tag="nrm2")
            nc.scalar.activation(nrm2[:], ssq2[:], ACTF.Sqrt)
            rinv2 = spool.tile([128, c.K], F32, tag="rinv2")
            nc.vector.reciprocal(rinv2[:], nrm2[:])
            for k in range(c.K):
                nc.vector.tensor_scalar(g2[:, k, :], g2[:, k, :],
                                        rinv2[:, k:k + 1], None, op0=ALU.mult)
            nc.sync.dma_start(
                o_feat[j * 128:(j + 1) * 128, :, :].rearrange("p k f -> p (k f)"),
                g2[:].rearrange("p k f -> p (k f)"))

    nc.compile()
    return nc


_BUILT = {}


def _get_built(cfg: Cfg):
    key = (cfg.B, cfg.A, cfg.F, cfg.NC, cfg.K, cfg.KP)
    if key not in _BUILT:
        _BUILT[key] = build(cfg)
    return _BUILT[key]


def run(image_features, attr_features, cfg: Cfg, trace=False, **kw):
    c = cfg
    img = np.ascontiguousarray(np.asarray(image_features, dtype=np.float32))
    attr = np.ascontiguousarray(np.asarray(attr_features, dtype=np.float32))
    assert img.shape == (c.B, c.F) and attr.shape == (c.A, c.F)

    nc = _get_built(cfg)
    attr_pad = np.zeros((c.NC * 8192, c.F), dtype=np.float32)
    for r in range(c.NC):
        attr_pad[r * 8192: r * 8192 + c.SH] = attr[r * c.SH:(r + 1) * c.SH]
    # [MT, p(feat within kc), kc, j(img within tile)] contiguous per-partition
    img_tiled = np.ascontiguousarray(
        img.reshape(c.MT, 128, c.KC, 128).transpose(0, 3, 2, 1)
    ).reshape(c.MT, 128, c.KC * 128)
    attr_T = np.ascontiguousarray(attr.T)
    in_maps = []
    for r in range(c.NC):
        s0 = r * c.SH
        hb = (s0 + np.arange(c.NH, dtype=np.float32) * c.HW)[None, :].astype(np.float32)
        in_maps.append({
            "attr_t": np.ascontiguousarray(attr_T[:, s0:s0 + c.SH]),
            "attr_n": np.ascontiguousarray(attr[s0:s0 + c.SH]),
            "attr_full": attr_pad,
            "img_t": img_tiled,
            "img_rows": np.ascontiguousarray(img[r * c.G * 128:(r + 1) * c.G * 128]),
            "hbase": np.ascontiguousarray(np.repeat(hb, 128, axis=0)),
        })
    try:
        res = run_bass_kernel_spmd(nc, in_maps, core_ids=list(range(c.NC)),
                                   trace=trace, **kw)
    except Exception:
        # transient NRT_EXEC_UNIT_UNRECOVERABLE wedges recover on retry
        import time as _time
        _time.sleep(2.0)
        res = run_bass_kernel_spmd(nc, in_maps, core_ids=list(range(c.NC)),
                                   trace=trace, **kw)
    feat = np.concatenate([res.results[r]["o_feat"] for r in range(c.NC)], axis=0)
    scores = np.concatenate([res.results[r]["o_scores"] for r in range(c.NC)], axis=0)
    return (feat, scores), res


def kernel(image_features, attr_features):
    (feat, scores), _ = run(image_features, attr_features, Cfg())
    return (feat, scores)
